# revision 9
# baseline (speedup 1.0000x reference)
"""Trainium2 Bass kernel for nn_DecoderLSTM_noAttention (greedy decode LSTM).

Strategy (8 NeuronCores, SPMD):
- Vocab-sharded FC: each core holds a 4000-column slice of W_fc (transposed)
  and computes its slice of the logits each step (fp32, exact).
- The LSTM recurrence (gates) is replicated on every core over the full
  batch of 64 (fp32).
- Greedy argmax: per-core top-8 via DVE max/max_index on the fp32 logits,
  then a tiny AllGather of (best value, global id) per batch row; every core
  deterministically picks the global winner and uses it as the next token
  (indirect-DMA embedding gather).
- Batch-sharded phase 0: each core reduces its 8 batches of encoder_outputs,
  AllGather of the summary vectors, then h0/c0 = summary @ W_init.T + b.
- Output: each core writes logits [31, 64, 4000]; the host assembles the
  full [64, 32, 32000] (t=0 stays zero).
"""
import numpy as np

import concourse.bass as bass
import concourse.bacc as bacc
import concourse.tile as tile
from concourse import mybir
from concourse.bass_utils import run_bass_kernel_spmd
from concourse.masks import make_identity

F32 = mybir.dt.float32
I32 = mybir.dt.int32
U32 = mybir.dt.uint32

B = 64          # batch
H = 512         # hidden = embed
V = 32000       # vocab
T = 32          # max_len
NPIX = 196
NCORES = 8
BL = B // NCORES      # local batch (phase 0)
VL = V // NCORES      # local vocab slice
NSTEPS = T - 1

_CACHE = {}


def _build_nc(nsteps=NSTEPS, out_slots=NSTEPS):
    nc = bacc.Bacc("TRN2", target_bir_lowering=False, debug=False, num_devices=NCORES)

    # ---- DRAM parameters ----
    emb_d = nc.dram_tensor("emb", [V, H], F32, kind="ExternalInput")
    wcat_d = nc.dram_tensor("wcatT", [8, 128, 2048], F32, kind="ExternalInput")
    wfc_d = nc.dram_tensor("wfcT", [4, 128, VL], F32, kind="ExternalInput")
    winit_d = nc.dram_tensor("winitT", [4, 128, 1024], F32, kind="ExternalInput")
    bcat_d = nc.dram_tensor("bcat", [1, 2048], F32, kind="ExternalInput")
    bfc_d = nc.dram_tensor("bfc", [1, VL], F32, kind="ExternalInput")
    binit_d = nc.dram_tensor("binit", [1, 1024], F32, kind="ExternalInput")
    enc_d = nc.dram_tensor("enc", [13, 128, H], F32, kind="ExternalInput")
    blk_d = nc.dram_tensor("blkdiag", [128, 13 * 8], F32, kind="ExternalInput")
    tok0_d = nc.dram_tensor("tok0", [B, 1], I32, kind="ExternalInput")
    vbase_d = nc.dram_tensor("vbase", [B, 1], F32, kind="ExternalInput")

    out_d = nc.dram_tensor("logits", [out_slots, B, VL], F32, kind="ExternalOutput")

    with tile.TileContext(nc) as tc:
        import contextlib
        with contextlib.ExitStack() as ctx:
            const = ctx.enter_context(tc.tile_pool(name="const", bufs=1))
            work = ctx.enter_context(tc.tile_pool(name="work", bufs=1))
            hc = ctx.enter_context(tc.tile_pool(name="hc", bufs=2))
            lgp = ctx.enter_context(tc.tile_pool(name="lgp", bufs=1))
            small = ctx.enter_context(tc.tile_pool(name="small", bufs=2))
            ptr = ctx.enter_context(tc.tile_pool(name="ptr", bufs=1, space="PSUM"))
            pg = ctx.enter_context(tc.tile_pool(name="pg", bufs=1, space="PSUM"))
            pfc = ctx.enter_context(tc.tile_pool(name="pfc", bufs=1, space="PSUM"))
            dram = ctx.enter_context(tc.tile_pool(name="dram", bufs=2, space="DRAM"))
            dramsh = ctx.enter_context(
                tc.tile_pool(name="dramsh", bufs=2, space="DRAM"))

            # ---- constants / weights into SBUF ----
            ident = const.tile([B, B], F32)
            make_identity(nc, ident[:])
            ones1 = const.tile([1, B], F32)
            nc.vector.memset(ones1[:], 1.0)
            vb64 = const.tile([B, 1], F32)
            nc.sync.dma_start(vb64[:], vbase_d[:])

            wcat = []
            for k in range(8):
                w = const.tile([128, 2048], F32, tag=f"wcat{k}")
                nc.sync.dma_start(w[:], wcat_d[k])
                wcat.append(w)
            wfc = []
            for k in range(4):
                w = const.tile([128, VL], F32, tag=f"wfc{k}")
                nc.sync.dma_start(w[:], wfc_d[k])
                wfc.append(w)
            bcat = const.tile([1, 2048], F32)
            nc.sync.dma_start(bcat[:], bcat_d[:])
            bfc = const.tile([1, VL], F32)
            nc.sync.dma_start(bfc[:], bfc_d[:])
            binit = const.tile([1, 1024], F32)
            nc.sync.dma_start(binit[:], binit_d[:])
            blk = work.tile([128, 13 * 8], F32, tag="t1")
            nc.sync.dma_start(blk[:], blk_d[:])

            def transpose_to(src, dst_tile):
                """src: SBUF [B, 512] fp32 -> dst SBUF [128, 4*B] (feature-major)."""
                for k in range(4):
                    pt = ptr.tile([128, B], F32, tag="ptr")
                    nc.tensor.transpose(
                        out=pt[:], in_=src[:, k * 128:(k + 1) * 128],
                        identity=ident[:])
                    nc.scalar.copy(dst_tile[:, k * B:(k + 1) * B], pt[:])

            # ================= phase 0 =================
            psum0 = pg.tile([BL, H], F32, tag="pg")
            for k in range(13):
                et = work.tile([128, H], F32, tag="tng")
                nc.sync.dma_start(et[:], enc_d[k])
                nc.tensor.matmul(
                    psum0[:], lhsT=blk[:, k * 8:(k + 1) * 8], rhs=et[:],
                    start=(k == 0), stop=(k == 12))
            sums = work.tile([BL, H], F32, tag="sgo")
            nc.scalar.copy(sums[:], psum0[:])

            cc0_in = dram.tile([BL, H], F32, tag="cc0i")
            cc0_out = dramsh.tile([NCORES, BL, H], F32, addr_space="Shared",
                                  tag="cc0o")
            nc.sync.dma_start(cc0_in[:], sums[:])
            nc.gpsimd.collective_compute(
                "AllGather", mybir.AluOpType.bypass,
                replica_groups=[list(range(NCORES))],
                ins=[cc0_in[:]], outs=[cc0_out[:]])
            sumfull = work.tile([B, H], F32, tag="x")
            nc.sync.dma_start(sumfull[:], cc0_out[:].rearrange("c b h -> (c b) h"))

            sumT = work.tile([128, 4 * B], F32, tag="xT")
            transpose_to(sumfull, sumT)

            phc = pfc.tile([B, 1024], F32, tag="pfc")
            for k in range(4):
                wi = work.tile([128, 1024], F32, tag="sigif")
                nc.sync.dma_start(wi[:], winit_d[k])
                for n in range(2):
                    nc.tensor.matmul(
                        phc[:, n * 512:(n + 1) * 512],
                        lhsT=sumT[:, k * B:(k + 1) * B],
                        rhs=wi[:, n * 512:(n + 1) * 512],
                        start=(k == 0), stop=False)
            for n in range(2):
                nc.tensor.matmul(
                    phc[:, n * 512:(n + 1) * 512], lhsT=ones1[:],
                    rhs=binit[:, n * 512:(n + 1) * 512],
                    start=False, stop=True)
            h_cur = hc.tile([B, H], F32, tag="h")
            c_cur = hc.tile([B, H], F32, tag="c")
            nc.scalar.copy(h_cur[:], phc[:, 0:512])
            nc.scalar.copy(c_cur[:], phc[:, 512:1024])
            hT = work.tile([128, 4 * B], F32, tag="hT")
            transpose_to(h_cur, hT)

            tok = small.tile([B, 1], I32, tag="tok")
            nc.sync.dma_start(tok[:], tok0_d[:])

            # ================= decode steps =================
            for t in range(nsteps):
                # --- embedding gather + transpose ---
                x_sb = work.tile([B, H], F32, tag="x")
                nc.gpsimd.indirect_dma_start(
                    out=x_sb[:], out_offset=None, in_=emb_d[:],
                    in_offset=bass.IndirectOffsetOnAxis(ap=tok[:, :1], axis=0))
                xT = work.tile([128, 4 * B], F32, tag="xT")
                transpose_to(x_sb, xT)

                # --- gates = [x|h] @ Wcat.T + bias ---
                pgt = pg.tile([B, 2048], F32, tag="pg")
                for k in range(8):
                    lhsT = xT[:, (k % 4) * B:(k % 4 + 1) * B] if k < 4 else \
                        hT[:, (k - 4) * B:(k - 3) * B]
                    for n in range(4):
                        nc.tensor.matmul(
                            pgt[:, n * 512:(n + 1) * 512], lhsT=lhsT,
                            rhs=wcat[k][:, n * 512:(n + 1) * 512],
                            start=(k == 0), stop=False)
                for n in range(4):
                    nc.tensor.matmul(
                        pgt[:, n * 512:(n + 1) * 512], lhsT=ones1[:],
                        rhs=bcat[:, n * 512:(n + 1) * 512],
                        start=False, stop=True)

                # --- pointwise LSTM ---
                AF = mybir.ActivationFunctionType
                sig_if = work.tile([B, 1024], F32, tag="sigif")
                nc.scalar.activation(sig_if[:], pgt[:, 0:1024], AF.Sigmoid)
                tng = work.tile([B, 512], F32, tag="tng")
                nc.scalar.activation(tng[:], pgt[:, 1024:1536], AF.Tanh)
                sgo = work.tile([B, 512], F32, tag="sgo")
                nc.scalar.activation(sgo[:], pgt[:, 1536:2048], AF.Sigmoid)

                t1 = work.tile([B, 512], F32, tag="t1")
                nc.vector.tensor_mul(t1[:], sig_if[:, 0:512], tng[:])
                t2 = work.tile([B, 512], F32, tag="t2")
                nc.vector.tensor_mul(t2[:], sig_if[:, 512:1024], c_cur[:])
                c_new = hc.tile([B, H], F32, tag="c")
                nc.vector.tensor_add(c_new[:], t2[:], t1[:])
                tc2 = work.tile([B, 512], F32, tag="tc2")
                nc.scalar.activation(tc2[:], c_new[:], AF.Tanh)
                h_new = hc.tile([B, H], F32, tag="h")
                nc.vector.tensor_mul(h_new[:], sgo[:], tc2[:])
                c_cur = c_new
                h_cur = h_new

                hT = work.tile([128, 4 * B], F32, tag="hT")
                transpose_to(h_new, hT)

                # --- FC: logits slice [64, VL] ---
                logits = lgp.tile([B, VL], F32, tag="logits")
                NSPLIT = ((0, 512), (512, 488))
                for v in range(4):
                    pf = pfc.tile([B, 1000], F32, tag="pfc")
                    for k in range(4):
                        for (no, nw) in NSPLIT:
                            nc.tensor.matmul(
                                pf[:, no:no + nw],
                                lhsT=hT[:, k * B:(k + 1) * B],
                                rhs=wfc[k][:, v * 1000 + no:v * 1000 + no + nw],
                                start=(k == 0), stop=False)
                    for (no, nw) in NSPLIT:
                        nc.tensor.matmul(
                            pf[:, no:no + nw], lhsT=ones1[:],
                            rhs=bfc[:, v * 1000 + no:v * 1000 + no + nw],
                            start=False, stop=True)
                    nc.scalar.copy(logits[:, v * 1000:(v + 1) * 1000], pf[:])

                nc.sync.dma_start(out_d[t], logits[:])

                # --- local top-8 + global argmax exchange ---
                mx = small.tile([B, 8], F32, tag="mx")
                nc.vector.max(out=mx[:], in_=logits[:])
                mi = small.tile([B, 8], U32, tag="mi")
                nc.vector.max_index(out=mi[:], in_max=mx[:], in_values=logits[:])

                gidf = small.tile([B, 1], F32, tag="gidf")
                nc.vector.tensor_copy(gidf[:], mi[:, 0:1])
                pack = small.tile([B, 2], F32, tag="pack")
                nc.vector.tensor_copy(pack[:, 0:1], mx[:, 0:1])
                nc.vector.tensor_add(pack[:, 1:2], gidf[:], vb64[:])

                cc_in = dram.tile([B, 2], F32, tag="cci")
                cc_out = dramsh.tile([NCORES, B, 2], F32, addr_space="Shared",
                                     tag="cco")
                nc.sync.dma_start(cc_in[:], pack[:])
                nc.gpsimd.collective_compute(
                    "AllGather", mybir.AluOpType.bypass,
                    replica_groups=[list(range(NCORES))],
                    ins=[cc_in[:]], outs=[cc_out[:]])

                arr = small.tile([B, 16], F32, tag="arr")
                nc.sync.dma_start(
                    arr[:, :].rearrange("b (c j) -> b c j", j=2),
                    cc_out[:].rearrange("c b j -> b c j"))
                vals = arr[:, :].rearrange("b (c j) -> b c j", j=2)[:, :, 0]
                gids = arr[:, :].rearrange("b (c j) -> b c j", j=2)[:, :, 1]

                wmax = small.tile([B, 8], F32, tag="wmax")
                nc.vector.max(out=wmax[:], in_=vals)
                msk = small.tile([B, 8], F32, tag="msk")
                nc.vector.tensor_scalar(
                    out=msk[:], in0=vals, scalar1=wmax[:, 0:1], scalar2=None,
                    op0=mybir.AluOpType.is_equal)
                gneg = small.tile([B, 8], F32, tag="gneg")
                nc.vector.tensor_scalar(
                    out=gneg[:], in0=gids, scalar1=-1.0, scalar2=40000.0,
                    op0=mybir.AluOpType.mult, op1=mybir.AluOpType.add)
                gsel = small.tile([B, 8], F32, tag="gsel")
                nc.vector.tensor_mul(gsel[:], msk[:], gneg[:])
                w2 = small.tile([B, 8], F32, tag="w2")
                nc.vector.max(out=w2[:], in_=gsel[:])
                tokf = small.tile([B, 1], F32, tag="tokf")
                nc.vector.tensor_scalar(
                    out=tokf[:], in0=w2[:, 0:1], scalar1=-1.0, scalar2=40000.0,
                    op0=mybir.AluOpType.mult, op1=mybir.AluOpType.add)
                tok = small.tile([B, 1], I32, tag="tok")
                nc.vector.tensor_copy(tok[:], tokf[:])

    nc.compile()
    return nc


def _prep_inputs(inputs):
    enc = np.ascontiguousarray(np.asarray(inputs["encoder_outputs"], np.float32))
    captions = np.asarray(inputs["captions"])
    emb = np.ascontiguousarray(np.asarray(inputs["embedding"], np.float32))
    W_ih = np.asarray(inputs["W_ih"], np.float32)
    b_ih = np.asarray(inputs["b_ih"], np.float32)
    W_hh = np.asarray(inputs["W_hh"], np.float32)
    b_hh = np.asarray(inputs["b_hh"], np.float32)
    W_fc = np.asarray(inputs["W_fc"], np.float32)
    b_fc = np.asarray(inputs["b_fc"], np.float32)
    W_init_h = np.asarray(inputs["W_init_h"], np.float32)
    b_init_h = np.asarray(inputs["b_init_h"], np.float32)
    W_init_c = np.asarray(inputs["W_init_c"], np.float32)
    b_init_c = np.asarray(inputs["b_init_c"], np.float32)

    wcatT = np.ascontiguousarray(
        np.concatenate([W_ih, W_hh], axis=1).T.reshape(8, 128, 2048))
    winitT = np.ascontiguousarray(
        (np.concatenate([W_init_h, W_init_c], axis=0) / np.float32(NPIX))
        .T.reshape(4, 128, 1024))
    bcat = (b_ih + b_hh).reshape(1, 2048)
    binit = np.concatenate([b_init_h, b_init_c]).reshape(1, 1024)
    tok0 = np.ascontiguousarray(captions[:, 0].astype(np.int32).reshape(B, 1))

    # block-diagonal ones for the per-batch encoder sum (13 chunks of 128 rows
    # over 8*196=1568 padded to 1664)
    blk = np.zeros((128, 13 * 8), np.float32)
    for k in range(13):
        for i in range(128):
            r = k * 128 + i
            if r < BL * NPIX:
                blk[i, k * 8 + r // NPIX] = 1.0

    in_maps = []
    for c in range(NCORES):
        enc_c = enc[c * BL:(c + 1) * BL].reshape(BL * NPIX, H)
        enc_pad = np.zeros((13 * 128, H), np.float32)
        enc_pad[:BL * NPIX] = enc_c
        wfc_slice = W_fc[c * VL:(c + 1) * VL]
        in_maps.append({
            "emb": emb,
            "wcatT": wcatT,
            "wfcT": np.ascontiguousarray(wfc_slice.T.reshape(4, 128, VL)),
            "winitT": winitT,
            "bcat": bcat,
            "bfc": np.ascontiguousarray(b_fc[c * VL:(c + 1) * VL].reshape(1, VL)),
            "binit": binit,
            "enc": enc_pad.reshape(13, 128, H),
            "blkdiag": blk,
            "tok0": tok0,
            "vbase": np.full((B, 1), c * VL, np.float32),
        })
    return in_maps


def kernel(**inputs) -> np.ndarray:
    if "nc" not in _CACHE:
        _CACHE["nc"] = _build_nc()
    nc = _CACHE["nc"]
    in_maps = _prep_inputs(inputs)
    res = run_bass_kernel_spmd(nc, in_maps, list(range(NCORES)))
    out = np.zeros((B, T, V), np.float32)
    for c in range(NCORES):
        lg = res.results[c]["logits"]            # [31, 64, VL]
        out[:, 1:, c * VL:(c + 1) * VL] = lg.transpose(1, 0, 2)
    return out
